# revision 32
# baseline (speedup 1.0000x reference)
"""GraphTransformer (TransformerConv + mean-pool) on 8 trn2 NeuronCores.

Strategy (two launches, nodes sharded 8 ways):
  Launch A (per core, 6250 nodes + pad -> 6272), TRANSPOSED output hT[ch, node]:
      out channels 0-511 q, 512-1023 k (fp8 DoubleRow matmuls, 2x TensorE),
      1024-1535 v, 1536-1599 skip (bf16 matmuls - weight-quantization error
      on v/skip hits the output directly, q/k error washes out in softmax).
      Bias + fp8-descale ride the ScalarE PSUM->SBUF copy (per-partition bias
      in the transposed layout).
  Host: build per-edge gathers grouped by dst tile (128 dst nodes, 9 chunks
      of 128 edge slots): qgT/kgT in channel-transposed layout [c, e] fp8,
      vg in edge-major [e, c] fp8, one-hot ind fp8.
  Launch B (per core, 49 dst tiles x 9 chunks of 128 edges):
      cast-DMA fp8->bf16 into SBUF (HBM traffic halved, DVE stays 2x)
      qkT[c,e] = qgT*kgT              (DVE 2x)
      s[e,h]   = sum_c qkT            (TensorE matmuls vs head-selector)
      w        = exp(s*scale) expand  (ScalarE, broadcast-read)
      wv       = w*vg                 (DVE 2x)
      num[d,:] += ind^T @ wv; den[d,h] += ind^T @ w   (TensorE)
      out[d,:] = mean_h(num/den) + skip[d,:]          (epilogue)
      pooled[g,:] += indng^T @ out    (TensorE, per-graph partial)
  Host: sum partial pooled over cores, divide by graph node counts.
"""

import numpy as np
import ml_dtypes

import concourse.bass as bass
from concourse import bacc
import concourse.mybir as mybir
import concourse.tile as tile
from concourse import bass_utils
from concourse.bass import ts

BF16 = mybir.dt.bfloat16
F32 = mybir.dt.float32
FP8 = mybir.dt.float8e4
NP_BF16 = ml_dtypes.bfloat16
NP_FP8 = ml_dtypes.float8_e4m3

N, E, B = 50000, 400000, 64
IN_DIM, OUT_DIM, HEADS = 768, 64, 8
HC = HEADS * OUT_DIM  # 512
NCORES = 8
TILES = 50  # dst tiles per core
NPAD = TILES * 128  # 6400 node slots per core (nodes assigned by degree-packing)
CHUNKS = 8  # edge chunks (of 128) per dst tile
CAP = CHUNKS * 128  # 1024 edge slots per tile
KCH = IN_DIM // 128  # 6 contraction chunks

SW = 32.0  # fp8 scale for the fused QK weights (escape subnormals)
SQ = 8.0  # fp8 scale for gathered q/k/v values
EXPSCALE = 0.125 / (SQ * SQ)  # 1/sqrt(64) with q,k prescale folded in

# launch A channel tiles: 8 x 128 QK (fp8) + 4 x 128 V (bf16) + 64 skip (bf16)
CHT = [(i * 128, 128, True) for i in range(8)] + [
    (1024 + i * 128, 128, False) for i in range(4)
] + [(1536, 64, False)]
SEGS = [(i * 512, 512) for i in range(12)] + [(6144, 256)]

TRACE = False
LAST_EXEC_NS = {}

_cache = {}


def _build_launch_a():
    nc = bacc.Bacc("TRN2", debug=False, num_devices=NCORES)
    x8 = nc.dram_tensor("x8", [128, KCH, NPAD], FP8, kind="ExternalInput").ap()
    x16 = nc.dram_tensor("x16", [128, KCH, NPAD], BF16, kind="ExternalInput").ap()
    w8 = nc.dram_tensor("w8", [128, KCH, 1024], FP8, kind="ExternalInput").ap()
    w16 = nc.dram_tensor("w16", [128, KCH, 576], BF16, kind="ExternalInput").ap()
    bT = nc.dram_tensor("bT", [128, 13], F32, kind="ExternalInput").ap()
    hT8 = nc.dram_tensor("hT8", [1536, NPAD], FP8, kind="ExternalOutput").ap()
    skipT = nc.dram_tensor("skipT", [64, NPAD], BF16, kind="ExternalOutput").ap()

    with tile.TileContext(nc) as tc:
        with (
            tc.tile_pool(name="const", bufs=1) as cpool,
            tc.tile_pool(name="ps", bufs=4, space="PSUM") as pspool,
            tc.tile_pool(name="outp", bufs=4) as outp,
        ):
            x8_sb = cpool.tile([128, KCH * NPAD], FP8)
            x16_sb = cpool.tile([128, KCH * NPAD], BF16)
            w8_sb = cpool.tile([128, KCH * 1024], FP8)
            w16_sb = cpool.tile([128, KCH * 576], BF16)
            bT_sb = cpool.tile([128, 13], F32)
            nc.sync.dma_start(bT_sb[:], bT[:, :])
            nc.sync.dma_start(w8_sb[:], w8[:, :, :])
            nc.sync.dma_start(w16_sb[:], w16[:, :, :])
            # deliver x in seg-blocks of all k-chunks: the first PSUM tile can
            # complete (and its ScalarE copy start) after ~1.2MB instead of 16MB
            nblk = 4
            for b0 in range(nblk):
                lo = (NPAD // nblk) * b0
                hi = (NPAD // nblk) * (b0 + 1) if b0 < nblk - 1 else NPAD
                for k in range(KCH):
                    nc.sync.dma_start(
                        x8_sb[:, k * NPAD + lo:k * NPAD + hi], x8[:, k, lo:hi])
            for b0 in range(nblk):
                lo = (NPAD // nblk) * b0
                hi = (NPAD // nblk) * (b0 + 1) if b0 < nblk - 1 else NPAD
                for k in range(KCH):
                    nc.sync.dma_start(
                        x16_sb[:, k * NPAD + lo:k * NPAD + hi], x16[:, k, lo:hi])
            x8r = x8_sb[:].rearrange("p (k m) -> p k m", k=KCH)
            x16r = x16_sb[:].rearrange("p (k m) -> p k m", k=KCH)
            w8r = w8_sb[:].rearrange("p (k n) -> p k n", k=KCH)
            w16r = w16_sb[:].rearrange("p (k n) -> p k n", k=KCH)

            for ch0, cw, is8 in CHT:
                for s0, sw in SEGS:
                    ps = pspool.tile([128, 512], F32, tag="ps")
                    if is8:
                        for p in range(KCH // 2):
                            nc.tensor.matmul(
                                ps[:cw, :sw],
                                lhsT=w8r[:, 2 * p:2 * p + 2, ch0:ch0 + cw],
                                rhs=x8r[:, 2 * p:2 * p + 2, s0:s0 + sw],
                                start=(p == 0),
                                stop=(p == KCH // 2 - 1),
                                perf_mode=mybir.MatmulPerfMode.DoubleRow,
                            )
                    else:
                        for k in range(KCH):
                            nc.tensor.matmul(
                                ps[:cw, :sw],
                                lhsT=w16r[:, k, ch0 - 1024:ch0 - 1024 + cw],
                                rhs=x16r[:, k, s0:s0 + sw],
                                start=(k == 0),
                                stop=(k == KCH - 1),
                            )
                    cidx = CHT.index((ch0, cw, is8))
                    is_skip = ch0 == 1536
                    h_sb = outp.tile(
                        [128, 512], BF16 if is_skip else FP8, tag="h16" if is_skip else "h8")
                    # qk tiles: out = psum*(SQ/SW) + SQ*b; v: psum*SQ + SQ*b; skip: psum + b
                    nc.scalar.activation(
                        h_sb[:cw, :sw],
                        ps[:cw, :sw],
                        func=mybir.ActivationFunctionType.Identity,
                        bias=bT_sb[:cw, cidx:cidx + 1],
                        scale=(SQ / SW) if is8 else (1.0 if is_skip else SQ),
                    )
                    if is_skip:
                        nc.sync.dma_start(skipT[:cw, s0:s0 + sw], h_sb[:cw, :sw])
                    else:
                        nc.sync.dma_start(hT8[ch0:ch0 + cw, s0:s0 + sw], h_sb[:cw, :sw])
    nc.compile()
    return nc


def _build_launch_b():
    nc = bacc.Bacc("TRN2", debug=False, num_devices=NCORES)
    qgT8 = nc.dram_tensor("qgT8", [TILES, 128, CHUNKS * 512], FP8, kind="ExternalInput").ap()
    kgT8 = nc.dram_tensor("kgT8", [TILES, 128, CHUNKS * 512], FP8, kind="ExternalInput").ap()
    vgA8 = nc.dram_tensor("vgA8", [TILES, 128, CHUNKS * 192], FP8, kind="ExternalInput").ap()
    vgB8 = nc.dram_tensor("vgB8", [TILES, 128, CHUNKS * 320], FP8, kind="ExternalInput").ap()
    ind8 = nc.dram_tensor("ind8", [TILES, 128, CHUNKS * 128], FP8, kind="ExternalInput").ap()
    skip16 = nc.dram_tensor("skip16", [TILES, 128, OUT_DIM], BF16, kind="ExternalInput").ap()
    indng16 = nc.dram_tensor("indng16", [TILES, 128, B], BF16, kind="ExternalInput").ap()
    isel16 = nc.dram_tensor("isel16", [128, 2], BF16, kind="ExternalInput").ap()
    pooled = nc.dram_tensor("pooled", [B, OUT_DIM], F32, kind="ExternalOutput").ap()

    with tile.TileContext(nc) as tc:
        with (
            tc.tile_pool(name="const", bufs=1) as cpool,
            tc.tile_pool(name="io", bufs=2) as iop,
            tc.tile_pool(name="work", bufs=3) as wp,
            tc.tile_pool(name="psS", bufs=3, space="PSUM") as psS,
            tc.tile_pool(name="psN", bufs=2, space="PSUM") as psN,
            tc.tile_pool(name="psD", bufs=2, space="PSUM") as psD,
            tc.tile_pool(name="psP", bufs=1, space="PSUM") as psP,
            tc.tile_pool(name="outp", bufs=1) as outp,
        ):
            isel_sb = cpool.tile([128, 2], BF16)
            nc.sync.dma_start(isel_sb[:], isel16[:, :])
            pool_ps = psP.tile([B, OUT_DIM], F32)
            for t0 in range(0, TILES, 2):
                gs = min(2, TILES - t0)
                # batch cast-DMAs over 2 tiles: bigger transfers, half the
                # SWDGE fixed costs
                qgT_sb = iop.tile([128, 2 * CHUNKS * 512], BF16, tag="qgT")
                kgT_sb = iop.tile([128, 2 * CHUNKS * 512], BF16, tag="kgT")
                # v split: heads 0-2 upcast to bf16 (DVE 2x), heads 3-7 stay
                # fp8 in SBUF (DVE 1x) - trades SBUF-write DMA for DVE slack
                vg16_sb = iop.tile([128, 2 * CHUNKS * 192], BF16, tag="vg16")
                vg8_sb = iop.tile([128, 2 * CHUNKS * 320], FP8, tag="vg8")
                ind2_sb = iop.tile([128, 2 * CHUNKS * 128], BF16, tag="ind")
                skip2_sb = iop.tile([128, 2 * OUT_DIM], BF16, tag="skip")
                indng2_sb = iop.tile([128, 2 * B], BF16, tag="indng")
                nc.gpsimd.dma_start(
                    qgT_sb[:, :gs * CHUNKS * 512],
                    qgT8[t0:t0 + gs].rearrange("t p j -> p t j"))
                nc.gpsimd.dma_start(
                    kgT_sb[:, :gs * CHUNKS * 512],
                    kgT8[t0:t0 + gs].rearrange("t p j -> p t j"))
                nc.gpsimd.dma_start(
                    vg16_sb[:, :gs * CHUNKS * 192],
                    vgA8[t0:t0 + gs].rearrange("t p j -> p t j"))
                nc.sync.dma_start(
                    vg8_sb[:, :gs * CHUNKS * 320],
                    vgB8[t0:t0 + gs].rearrange("t p j -> p t j"))
                nc.gpsimd.dma_start(
                    ind2_sb[:, :gs * CHUNKS * 128],
                    ind8[t0:t0 + gs].rearrange("t p j -> p t j"))
                nc.sync.dma_start(
                    skip2_sb[:, :gs * OUT_DIM],
                    skip16[t0:t0 + gs].rearrange("t p j -> p t j"))
                nc.sync.dma_start(
                    indng2_sb[:, :gs * B],
                    indng16[t0:t0 + gs].rearrange("t p j -> p t j"))
                for g in range(gs):
                    t = t0 + g
                    _launch_b_tile_body(
                        nc, wp, psS, psN, psD, isel_sb, pool_ps, t,
                        qgT_sb[:, g * CHUNKS * 512:(g + 1) * CHUNKS * 512],
                        kgT_sb[:, g * CHUNKS * 512:(g + 1) * CHUNKS * 512],
                        vg16_sb[:, g * CHUNKS * 192:(g + 1) * CHUNKS * 192],
                        vg8_sb[:, g * CHUNKS * 320:(g + 1) * CHUNKS * 320],
                        ind2_sb[:, g * CHUNKS * 128:(g + 1) * CHUNKS * 128],
                        skip2_sb[:, g * OUT_DIM:(g + 1) * OUT_DIM],
                        indng2_sb[:, g * B:(g + 1) * B],
                    )
            pooled_sb = outp.tile([B, OUT_DIM], F32)
            nc.vector.tensor_copy(pooled_sb[:], pool_ps[:])
            nc.sync.dma_start(pooled[:], pooled_sb[:])
    nc.compile()
    return nc


def _launch_b_tile_body(nc, wp, psS, psN, psD, isel_sb, pool_ps, t,
                        qgT_ap, kgT_ap, vg16_ap, vg8_ap, ind_ap, skip_ap,
                        indng_ap):
    q4 = qgT_ap.rearrange("p (ch cb e) -> p ch cb e", ch=CHUNKS, cb=4)
    k4 = kgT_ap.rearrange("p (ch cb e) -> p ch cb e", ch=CHUNKS, cb=4)

    if True:
            if True:
                num_ps = psN.tile([128, HC], F32, tag="num")
                den_ps = psD.tile([128, HEADS], F32, tag="den")
                for c0 in range(0, CHUNKS, 2):
                    w2 = min(2, CHUNKS - c0)
                    qkT = wp.tile([128, 2 * 512], BF16, tag="qkT")
                    qk4 = qkT[:].rearrange("p (w cb e) -> p w cb e", w=2, cb=4)
                    nc.vector.tensor_mul(
                        qk4[:, :w2], q4[:, c0:c0 + w2], k4[:, c0:c0 + w2])
                    s_ps = psS.tile([128, 2 * HEADS], F32, tag="s")
                    for j in range(w2):
                        for cb in range(4):
                            nc.tensor.matmul(
                                s_ps[:, j * HEADS + 2 * cb: j * HEADS + 2 * cb + 2],
                                lhsT=qk4[:, j, cb, :],
                                rhs=isel_sb[:],
                                start=True,
                                stop=True,
                            )
                    wexp = wp.tile([128, 2 * 512], BF16, tag="wexp")
                    nc.scalar.activation(
                        out=wexp[:, :w2 * 512].rearrange("p (h c) -> p h c", h=w2 * HEADS),
                        in_=s_ps[:, :w2 * HEADS].rearrange("p h -> p h ()").to_broadcast(
                            [128, w2 * HEADS, OUT_DIM]),
                        func=mybir.ActivationFunctionType.Exp,
                        scale=float(EXPSCALE),
                    )
                    wv = wp.tile([128, 2 * 512], BF16, tag="wv")
                    wvr = wv[:].rearrange("p (w hc) -> p w hc", w=2)
                    wer = wexp[:].rearrange("p (w hc) -> p w hc", w=2)
                    v16r = vg16_ap.rearrange("p (ch x) -> p ch x", ch=CHUNKS)
                    v8r = vg8_ap.rearrange("p (ch x) -> p ch x", ch=CHUNKS)
                    nc.vector.tensor_mul(
                        wvr[:, :w2, 0:192], wer[:, :w2, 0:192],
                        v16r[:, c0:c0 + w2])
                    nc.vector.tensor_mul(
                        wvr[:, :w2, 192:512], wer[:, :w2, 192:512],
                        v8r[:, c0:c0 + w2])
                    for j in range(w2):
                        c = c0 + j
                        nc.tensor.matmul(
                            num_ps[:], lhsT=ind_ap[:, ts(c, 128)], rhs=wv[:, ts(j, 512)],
                            start=(c == 0), stop=(c == CHUNKS - 1),
                        )
                        nc.tensor.matmul(
                            den_ps[:], lhsT=ind_ap[:, ts(c, 128)],
                            rhs=wexp[:, ts(j, 512)].rearrange("p (h c) -> p h c", h=HEADS)[:, :, 0],
                            start=(c == 0), stop=(c == CHUNKS - 1),
                        )
                # epilogue: out = mean_h(num/den)/SQ + skip
                rec = wp.tile([128, HEADS], F32, tag="rec")
                nc.vector.tensor_scalar(
                    out=rec[:], in0=den_ps[:],
                    scalar1=float(HEADS * SQ), scalar2=1e-12,
                    op0=mybir.AluOpType.mult, op1=mybir.AluOpType.add,
                )
                nc.vector.reciprocal(rec[:], rec[:])
                recx = wp.tile([128, HC], BF16, tag="recx")
                nc.scalar.activation(
                    out=recx[:].rearrange("p (h c) -> p h c", h=HEADS),
                    in_=rec[:].rearrange("p h -> p h ()").to_broadcast(
                        [128, HEADS, OUT_DIM]),
                    func=mybir.ActivationFunctionType.Copy,
                )
                num_sb = wp.tile([128, HC], BF16, tag="numsb")
                nc.scalar.activation(
                    out=num_sb[:], in_=num_ps[:],
                    func=mybir.ActivationFunctionType.Copy,
                )
                mh = wp.tile([128, HC], BF16, tag="mh")
                nc.vector.tensor_mul(mh[:], num_sb[:], recx[:])
                f1 = wp.tile([128, 256], BF16, tag="f1")
                nc.vector.tensor_add(f1[:], mh[:, :256], mh[:, 256:])
                f2 = wp.tile([128, 128], BF16, tag="f2")
                nc.vector.tensor_add(f2[:], f1[:, :128], f1[:, 128:])
                f3 = wp.tile([128, 64], BF16, tag="f3")
                nc.vector.tensor_add(f3[:], f2[:, :64], f2[:, 64:])
                out_sb = wp.tile([128, OUT_DIM], BF16, tag="outsb")
                nc.vector.tensor_add(out_sb[:], f3[:], skip_ap[:])
                nc.tensor.matmul(
                    pool_ps[:], lhsT=indng_ap[:], rhs=out_sb[:],
                    start=(t == 0), stop=(t == TILES - 1),
                )


def _get_programs():
    if "A" not in _cache:
        _cache["A"] = _build_launch_a()
    if "B" not in _cache:
        _cache["B"] = _build_launch_b()
    return _cache["A"], _cache["B"]


LAST_TRACE_PATH = {}


def _ensure_hook_shim():
    import sys
    import types

    if "antenv.axon_hooks" in sys.modules:
        return
    mod = types.ModuleType("antenv.axon_hooks")
    holder = [None]
    mod.set_axon_ntff_profile_hook = lambda h: holder.__setitem__(0, h)
    mod.get_axon_ntff_profile_hook = lambda: holder[0]
    sys.modules["antenv.axon_hooks"] = mod
    import antenv

    antenv.axon_hooks = mod
    from trn_agent_boot.trn_boot import _ntff_profile_via_ctypes

    mod.set_axon_ntff_profile_hook(
        _ntff_profile_via_ctypes("/opt/axon/libaxon_pjrt.so")
    )


def _run(nc, in_maps, label):
    if not TRACE:
        res = bass_utils.run_bass_kernel_spmd(nc, in_maps, list(range(NCORES)))
        return res.results

    import glob
    import os
    import tempfile

    from concourse import bass2jax
    from concourse._compat import FishPath
    import gauge.profiler

    _ensure_hook_shim()
    import antenv.axon_hooks as hooks

    tmpdir = tempfile.mkdtemp(prefix=f"bass_{label}_")
    with hooks.get_axon_ntff_profile_hook()(tmpdir, [0]):
        results = bass2jax.run_bass_via_pjrt(nc, in_maps, n_cores=NCORES)
    exec_ns = None
    try:
        ntffs = glob.glob(os.path.join(tmpdir, "*_body*.ntff"))
        if ntffs:
            profile = gauge.profiler.Profile(
                profile_path=FishPath(tmpdir),
                kernel_dev_mode=True,
                profile_on_exit=False,
                bass_kernel=nc.m,
                offline_processing=True,
                fname="*_body*",
            )
            prs = profile.to_perfetto(model_index=(0,))
            if prs:
                exec_ns = max(p.exec_time_ns for p in prs)
                LAST_TRACE_PATH[label] = (tmpdir, [p.trace_path for p in prs])
        else:
            print(f"[{label}] no ntff files in {tmpdir}: {os.listdir(tmpdir)}")
    except Exception as e:  # profiling must never break the run
        print(f"[{label}] profile processing failed: {type(e).__name__}: {e}")
    LAST_EXEC_NS[label] = exec_ns
    return results


def _to_fp8(a):
    return np.clip(a, -240.0, 240.0).astype(NP_FP8)


def kernel(x, edge_index, batch, W_emb, b_emb, Wq, bq, Wk, bk, Wv, bv, Wskip, bskip):
    x = np.asarray(x, np.float32)
    edge_index = np.asarray(edge_index)
    batch_np = np.asarray(batch)
    ncA, ncB = _get_programs()

    # ---- host prep for launch A: fold W_emb/b_emb into the qkv/skip weights ----
    wemb_f = np.asarray(W_emb, np.float32)
    bemb_f = np.asarray(b_emb, np.float32)
    wqk = np.concatenate([np.asarray(Wq, np.float32), np.asarray(Wk, np.float32)], axis=1)
    wvs = np.concatenate([np.asarray(Wv, np.float32), np.asarray(Wskip, np.float32)], axis=1)
    bqk = np.concatenate([np.asarray(bq, np.float32), np.asarray(bk, np.float32)])
    bvs = np.concatenate([np.asarray(bv, np.float32), np.asarray(bskip, np.float32)])
    w8_f = (wemb_f @ wqk) * SW  # [768, 1024]
    w16_f = wemb_f @ wvs  # [768, 576]
    b8 = bemb_f @ wqk + bqk  # [1024]
    b16 = bemb_f @ wvs + bvs  # [576]

    w8_np = np.ascontiguousarray(
        _to_fp8(w8_f).reshape(KCH, 128, 1024).transpose(1, 0, 2))
    w16_np = np.ascontiguousarray(
        w16_f.astype(NP_BF16).reshape(KCH, 128, 576).transpose(1, 0, 2))
    bT_np = np.zeros((128, 13), np.float32)
    for i in range(8):
        bT_np[:, i] = SQ * b8[i * 128:(i + 1) * 128]
    for i in range(4):
        bT_np[:, 8 + i] = SQ * b16[i * 128:(i + 1) * 128]
    bT_np[:64, 12] = b16[512:576]

    # ---- degree-balanced node -> (core, tile, slot) assignment ----
    # snake-pack nodes by in-degree over all 400 tiles so every tile's edge
    # load fits CHUNKS*128 slots; repair pass for rare overloads
    src = np.asarray(edge_index[0], np.int64)
    dst = np.asarray(edge_index[1], np.int64)
    deg = np.bincount(dst, minlength=N).astype(np.int64)
    ntile = NCORES * TILES  # 400
    order_n = np.argsort(-deg, kind="stable")
    ridx = np.arange(N) // ntile
    posr = np.arange(N) % ntile
    tile_sorted = np.where(ridx % 2 == 0, posr, ntile - 1 - posr)
    tile_of_node = np.empty(N, np.int64)
    tile_of_node[order_n] = tile_sorted
    loads = np.bincount(tile_of_node, weights=deg.astype(np.float64),
                        minlength=ntile).astype(np.int64)
    cnt_t = np.bincount(tile_of_node, minlength=ntile)
    for _ in range(2000):
        tmax = int(loads.argmax())
        if loads[tmax] <= CAP:
            break
        tmin = int(np.where(cnt_t < 128, loads, np.iinfo(np.int64).max).argmin())
        members = np.where(tile_of_node == tmax)[0]
        need = loads[tmax] - CAP
        dm = deg[members]
        ok = dm >= need
        nmove = members[np.where(ok, dm, np.iinfo(np.int64).max).argmin()] \
            if ok.any() else members[dm.argmax()]
        tile_of_node[nmove] = tmin
        loads[tmax] -= deg[nmove]
        loads[tmin] += deg[nmove]
        cnt_t[tmax] -= 1
        cnt_t[tmin] += 1
    order2 = np.argsort(tile_of_node, kind="stable")
    g_sorted = tile_of_node[order2]
    starts_t = np.searchsorted(g_sorted, np.arange(ntile))
    slot_sorted = np.arange(N) - starts_t[g_sorted]
    pos_sorted = (g_sorted // TILES) * NPAD + (g_sorted % TILES) * 128 + slot_sorted
    pos_of_node = np.empty(N, np.int64)
    pos_of_node[order2] = pos_sorted

    xflat = np.zeros((NCORES * NPAD, IN_DIM), np.float32)
    xflat[pos_of_node] = x
    xpad = xflat.reshape(NCORES, NPAD, IN_DIM)
    in_maps_a = []
    for c in range(NCORES):
        xT = xpad[c].T  # [768, 6272]
        xTk = xT.reshape(KCH, 128, NPAD).transpose(1, 0, 2)  # [128, 6, 6272]
        in_maps_a.append({
            "x8": np.ascontiguousarray(_to_fp8(xTk)),
            "x16": np.ascontiguousarray(xTk.astype(NP_BF16)),
            "w8": w8_np, "w16": w16_np, "bT": bT_np,
        })
    res_a = _run(ncA, in_maps_a, "A")

    # ---- host mid: assemble qT/kT/v/skip and build edge-sorted gathers ----
    # global padded node index: n -> (n // NPC) * NPAD + n % NPC
    qT8 = np.concatenate(
        [res_a[c]["hT8"][0:512] for c in range(NCORES)], axis=1)  # [512, 8*NPAD]
    kT8 = np.concatenate(
        [res_a[c]["hT8"][512:1024] for c in range(NCORES)], axis=1)
    v8 = np.concatenate(
        [res_a[c]["hT8"][1024:1536].T for c in range(NCORES)], axis=0)  # [8*NPAD, 512]

    dstp = pos_of_node[dst]
    loc = dstp % NPAD
    tile_g = (dstp // NPAD) * TILES + loc // 128  # 0 .. 8*50-1
    dloc = loc % 128
    order = np.argsort(tile_g, kind="stable")
    tg_s = tile_g[order]
    counts = np.bincount(tg_s, minlength=ntile)
    if counts.max() > CAP:
        raise RuntimeError(f"tile capacity exceeded: {counts.max()} > {CAP}")
    starts = np.zeros(ntile, np.int64)
    starts[1:] = np.cumsum(counts)[:-1]
    pos = np.arange(E) - starts[tg_s]
    rows = tg_s * CAP + pos  # slot in [ntile*CAP]

    srcp_pad = np.zeros(ntile * CAP, np.int64)
    srcp_pad[rows] = pos_of_node[src[order]]
    dst_pad = np.full(ntile * CAP, -1, np.int64)
    dst_pad[rows] = dloc[order]
    dstgp_pad = np.zeros(ntile * CAP, np.int64)
    dstgp_pad[rows] = dstp[order]

    def t_gather(mT8, idx):  # [512, npad*8] cols idx [ntile_c*CAP] -> [49,128,4608]
        g = mT8[:, idx]  # [512, 56448]
        g = g.reshape(4, 128, TILES, CHUNKS, 128).transpose(2, 1, 3, 0, 4)
        return np.ascontiguousarray(g.reshape(TILES, 128, CHUNKS * 512))

    def e_gather(m8, idx):  # [npad*8, 512] rows idx -> [49, 128, 4608]
        g = m8[idx]  # [56448, 512]
        g = g.reshape(TILES, CHUNKS, 128, 512).transpose(0, 2, 1, 3)
        return np.ascontiguousarray(g.reshape(TILES, 128, CHUNKS * 512))

    def tileize(a):  # [ntile_c*CAP, d] -> [49, 128, CHUNKS*d]
        d = a.shape[1]
        return np.ascontiguousarray(
            a.reshape(TILES, CHUNKS, 128, d).transpose(0, 2, 1, 3).reshape(
                TILES, 128, CHUNKS * d))

    batch_pad = np.full(NCORES * NPAD, -1, np.int64)
    batch_pad[pos_of_node] = batch_np
    indng_all = (batch_pad[:, None] == np.arange(B)[None, :]).astype(NP_BF16)
    indng_all = indng_all.reshape(NCORES, TILES, 128, B)

    isel_np = np.zeros((128, 2), NP_BF16)
    isel_np[:64, 0] = 1
    isel_np[64:, 1] = 1

    in_maps_b = []
    for c in range(NCORES):
        sl = slice(c * TILES * CAP, (c + 1) * TILES * CAP)
        skip_c = np.ascontiguousarray(
            res_a[c]["skipT"].T.reshape(TILES, 128, OUT_DIM))
        vg_full = e_gather(v8, srcp_pad[sl]).reshape(TILES, 128, CHUNKS, 512)
        in_maps_b.append({
            "qgT8": t_gather(qT8, dstgp_pad[sl]),
            "kgT8": t_gather(kT8, srcp_pad[sl]),
            "vgA8": np.ascontiguousarray(
                vg_full[..., :192].reshape(TILES, 128, CHUNKS * 192)),
            "vgB8": np.ascontiguousarray(
                vg_full[..., 192:].reshape(TILES, 128, CHUNKS * 320)),
            "ind8": tileize(
                (dst_pad[sl, None] == np.arange(128)[None, :]).astype(NP_FP8)),
            "skip16": skip_c,
            "indng16": indng_all[c],
            "isel16": isel_np,
        })
    res_b = _run(ncB, in_maps_b, "B")

    pooled = np.zeros((B, OUT_DIM), np.float64)
    for c in range(NCORES):
        pooled += res_b[c]["pooled"].astype(np.float64)
    cnt = np.bincount(batch_np, minlength=B).astype(np.float64)
    pooled /= np.maximum(cnt, 1.0)[:, None]
    return pooled.astype(np.float32)


# revision 39
# speedup vs baseline: 1.1761x; 1.1761x over previous
"""GraphTransformer (TransformerConv + mean-pool) on 8 trn2 NeuronCores.

Strategy (two launches, nodes sharded 8 ways):
  Launch A (per core, 6250 nodes + pad -> 6272), TRANSPOSED output hT[ch, node]:
      out channels 0-511 q, 512-1023 k (fp8 DoubleRow matmuls, 2x TensorE),
      1024-1535 v, 1536-1599 skip (bf16 matmuls - weight-quantization error
      on v/skip hits the output directly, q/k error washes out in softmax).
      Bias + fp8-descale ride the ScalarE PSUM->SBUF copy (per-partition bias
      in the transposed layout).
  Host: build per-edge gathers grouped by dst tile (128 dst nodes, 9 chunks
      of 128 edge slots): qgT/kgT in channel-transposed layout [c, e] fp8,
      vg in edge-major [e, c] fp8, one-hot ind fp8.
  Launch B (per core, 49 dst tiles x 9 chunks of 128 edges):
      cast-DMA fp8->bf16 into SBUF (HBM traffic halved, DVE stays 2x)
      qkT[c,e] = qgT*kgT              (DVE 2x)
      s[e,h]   = sum_c qkT            (TensorE matmuls vs head-selector)
      w        = exp(s*scale) expand  (ScalarE, broadcast-read)
      wv       = w*vg                 (DVE 2x)
      num[d,:] += ind^T @ wv; den[d,h] += ind^T @ w   (TensorE)
      out[d,:] = mean_h(num/den) + skip[d,:]          (epilogue)
      pooled[g,:] += indng^T @ out    (TensorE, per-graph partial)
  Host: sum partial pooled over cores, divide by graph node counts.
"""

import numpy as np
import ml_dtypes

import concourse.bass as bass
from concourse import bacc
import concourse.mybir as mybir
import concourse.tile as tile
from concourse import bass_utils
from concourse.bass import ts

BF16 = mybir.dt.bfloat16
F32 = mybir.dt.float32
FP8 = mybir.dt.float8e4
NP_BF16 = ml_dtypes.bfloat16
NP_FP8 = ml_dtypes.float8_e4m3

N, E, B = 50000, 400000, 64
IN_DIM, OUT_DIM, HEADS = 768, 64, 8
HC = HEADS * OUT_DIM  # 512
NCORES = 8
TILES = 50  # dst tiles per core
NPAD = TILES * 128  # 6400 node slots per core (nodes assigned by degree-packing)
CHUNKS = 8  # edge chunks (of 128) per dst tile
CAP = CHUNKS * 128  # 1024 edge slots per tile
KCH = IN_DIM // 128  # 6 contraction chunks

SW = 32.0  # fp8 scale for the fused QK weights (escape subnormals)
SQ = 8.0  # fp8 scale for gathered q/k/v values
EXPSCALE = 0.125 / (SQ * SQ)  # 1/sqrt(64) with q,k prescale folded in

# launch A channel tiles: 8 x 128 QK (fp8) + 4 x 128 V (bf16) + 64 skip (bf16)
CHT = [(i * 128, 128, True) for i in range(8)] + [
    (1024 + i * 128, 128, False) for i in range(4)
] + [(1536, 64, False)]
SEGS = [(i * 512, 512) for i in range(12)] + [(6144, 256)]

TRACE = False
LAST_EXEC_NS = {}

_cache = {}


def _build_launch_a():
    nc = bacc.Bacc("TRN2", debug=False, num_devices=NCORES)
    x8 = nc.dram_tensor("x8", [128, KCH, NPAD], FP8, kind="ExternalInput").ap()
    x16 = nc.dram_tensor("x16", [128, KCH, NPAD], BF16, kind="ExternalInput").ap()
    w8 = nc.dram_tensor("w8", [128, KCH, 1024], FP8, kind="ExternalInput").ap()
    w16 = nc.dram_tensor("w16", [128, KCH, 576], BF16, kind="ExternalInput").ap()
    bT = nc.dram_tensor("bT", [128, 13], F32, kind="ExternalInput").ap()
    hT8 = nc.dram_tensor("hT8", [1536, NPAD], FP8, kind="ExternalOutput").ap()
    skipT = nc.dram_tensor("skipT", [64, NPAD], BF16, kind="ExternalOutput").ap()

    with tile.TileContext(nc) as tc:
        with (
            tc.tile_pool(name="const", bufs=1) as cpool,
            tc.tile_pool(name="ps", bufs=4, space="PSUM") as pspool,
            tc.tile_pool(name="outp", bufs=4) as outp,
        ):
            x8_sb = cpool.tile([128, KCH * NPAD], FP8)
            x16_sb = cpool.tile([128, KCH * NPAD], BF16)
            w8_sb = cpool.tile([128, KCH * 1024], FP8)
            w16_sb = cpool.tile([128, KCH * 576], BF16)
            bT_sb = cpool.tile([128, 13], F32)
            nc.sync.dma_start(bT_sb[:], bT[:, :])
            nc.sync.dma_start(w8_sb[:], w8[:, :, :])
            nc.sync.dma_start(w16_sb[:], w16[:, :, :])
            # deliver x in seg-blocks of all k-chunks: the first PSUM tile can
            # complete (and its ScalarE copy start) after ~1.2MB instead of 16MB
            nblk = 4
            for b0 in range(nblk):
                lo = (NPAD // nblk) * b0
                hi = (NPAD // nblk) * (b0 + 1) if b0 < nblk - 1 else NPAD
                for k in range(KCH):
                    nc.sync.dma_start(
                        x8_sb[:, k * NPAD + lo:k * NPAD + hi], x8[:, k, lo:hi])
            for b0 in range(nblk):
                lo = (NPAD // nblk) * b0
                hi = (NPAD // nblk) * (b0 + 1) if b0 < nblk - 1 else NPAD
                for k in range(KCH):
                    nc.sync.dma_start(
                        x16_sb[:, k * NPAD + lo:k * NPAD + hi], x16[:, k, lo:hi])
            x8r = x8_sb[:].rearrange("p (k m) -> p k m", k=KCH)
            x16r = x16_sb[:].rearrange("p (k m) -> p k m", k=KCH)
            w8r = w8_sb[:].rearrange("p (k n) -> p k n", k=KCH)
            w16r = w16_sb[:].rearrange("p (k n) -> p k n", k=KCH)

            for ch0, cw, is8 in CHT:
                for s0, sw in SEGS:
                    ps = pspool.tile([128, 512], F32, tag="ps")
                    if is8:
                        for p in range(KCH // 2):
                            nc.tensor.matmul(
                                ps[:cw, :sw],
                                lhsT=w8r[:, 2 * p:2 * p + 2, ch0:ch0 + cw],
                                rhs=x8r[:, 2 * p:2 * p + 2, s0:s0 + sw],
                                start=(p == 0),
                                stop=(p == KCH // 2 - 1),
                                perf_mode=mybir.MatmulPerfMode.DoubleRow,
                            )
                    else:
                        for k in range(KCH):
                            nc.tensor.matmul(
                                ps[:cw, :sw],
                                lhsT=w16r[:, k, ch0 - 1024:ch0 - 1024 + cw],
                                rhs=x16r[:, k, s0:s0 + sw],
                                start=(k == 0),
                                stop=(k == KCH - 1),
                            )
                    cidx = CHT.index((ch0, cw, is8))
                    is_skip = ch0 == 1536
                    h_sb = outp.tile(
                        [128, 512], BF16 if is_skip else FP8, tag="h16" if is_skip else "h8")
                    # qk tiles: out = psum*(SQ/SW) + SQ*b; v: psum*SQ + SQ*b; skip: psum + b
                    nc.scalar.activation(
                        h_sb[:cw, :sw],
                        ps[:cw, :sw],
                        func=mybir.ActivationFunctionType.Identity,
                        bias=bT_sb[:cw, cidx:cidx + 1],
                        scale=(SQ / SW) if is8 else (1.0 if is_skip else SQ),
                    )
                    if is_skip:
                        nc.sync.dma_start(skipT[:cw, s0:s0 + sw], h_sb[:cw, :sw])
                    else:
                        nc.sync.dma_start(hT8[ch0:ch0 + cw, s0:s0 + sw], h_sb[:cw, :sw])
    nc.compile()
    return nc


def _build_launch_b():
    nc = bacc.Bacc("TRN2", debug=False, num_devices=NCORES)
    qgT8 = nc.dram_tensor("qgT8", [TILES, 128, CHUNKS * 512], FP8, kind="ExternalInput").ap()
    kgT8 = nc.dram_tensor("kgT8", [TILES, 128, CHUNKS * 512], FP8, kind="ExternalInput").ap()
    vg8 = nc.dram_tensor("vg8", [TILES, 128, CHUNKS * 512], FP8, kind="ExternalInput").ap()
    ind8 = nc.dram_tensor("ind8", [TILES, 128, CHUNKS * 128], FP8, kind="ExternalInput").ap()
    skip16 = nc.dram_tensor("skip16", [TILES, 128, OUT_DIM], BF16, kind="ExternalInput").ap()
    indng16 = nc.dram_tensor("indng16", [TILES, 128, B], BF16, kind="ExternalInput").ap()
    isel16 = nc.dram_tensor("isel16", [128, 2], BF16, kind="ExternalInput").ap()
    pooled = nc.dram_tensor("pooled", [B, OUT_DIM], F32, kind="ExternalOutput").ap()

    with tile.TileContext(nc) as tc:
        with (
            tc.tile_pool(name="const", bufs=1) as cpool,
            tc.tile_pool(name="io", bufs=2) as iop,
            tc.tile_pool(name="work", bufs=3) as wp,
            tc.tile_pool(name="psS", bufs=3, space="PSUM") as psS,
            tc.tile_pool(name="psN", bufs=2, space="PSUM") as psN,
            tc.tile_pool(name="psD", bufs=2, space="PSUM") as psD,
            tc.tile_pool(name="psP", bufs=1, space="PSUM") as psP,
            tc.tile_pool(name="outp", bufs=1) as outp,
        ):
            isel_sb = cpool.tile([128, 2], BF16)
            nc.sync.dma_start(isel_sb[:], isel16[:, :])
            pool_ps = psP.tile([B, OUT_DIM], F32)
            for t0 in range(0, TILES, 2):
                gs = min(2, TILES - t0)
                # batch cast-DMAs over 2 tiles: bigger transfers, half the
                # SWDGE fixed costs
                qgT_sb = iop.tile([128, 2 * CHUNKS * 512], BF16, tag="qgT")
                kgT_sb = iop.tile([128, 2 * CHUNKS * 512], BF16, tag="kgT")
                # vg stays fp8 in SBUF: the wv-mul reads the narrow exp output
                # broadcast (stride-0 -> 1x anyway), so fp8 costs DVE nothing
                vg2_sb = iop.tile([128, 2 * CHUNKS * 512], FP8, tag="vg")
                ind2_sb = iop.tile([128, 2 * CHUNKS * 128], BF16, tag="ind")
                skip2_sb = iop.tile([128, 2 * OUT_DIM], BF16, tag="skip")
                indng2_sb = iop.tile([128, 2 * B], BF16, tag="indng")
                nc.gpsimd.dma_start(
                    qgT_sb[:, :gs * CHUNKS * 512],
                    qgT8[t0:t0 + gs].rearrange("t p j -> p t j"))
                nc.gpsimd.dma_start(
                    kgT_sb[:, :gs * CHUNKS * 512],
                    kgT8[t0:t0 + gs].rearrange("t p j -> p t j"))
                nc.sync.dma_start(
                    vg2_sb[:, :gs * CHUNKS * 512],
                    vg8[t0:t0 + gs].rearrange("t p j -> p t j"))
                nc.gpsimd.dma_start(
                    ind2_sb[:, :gs * CHUNKS * 128],
                    ind8[t0:t0 + gs].rearrange("t p j -> p t j"))
                nc.sync.dma_start(
                    skip2_sb[:, :gs * OUT_DIM],
                    skip16[t0:t0 + gs].rearrange("t p j -> p t j"))
                nc.sync.dma_start(
                    indng2_sb[:, :gs * B],
                    indng16[t0:t0 + gs].rearrange("t p j -> p t j"))
                for g in range(gs):
                    t = t0 + g
                    _launch_b_tile_body(
                        nc, wp, psS, psN, psD, isel_sb, pool_ps, t,
                        qgT_sb[:, g * CHUNKS * 512:(g + 1) * CHUNKS * 512],
                        kgT_sb[:, g * CHUNKS * 512:(g + 1) * CHUNKS * 512],
                        vg2_sb[:, g * CHUNKS * 512:(g + 1) * CHUNKS * 512],
                        ind2_sb[:, g * CHUNKS * 128:(g + 1) * CHUNKS * 128],
                        skip2_sb[:, g * OUT_DIM:(g + 1) * OUT_DIM],
                        indng2_sb[:, g * B:(g + 1) * B],
                    )
            pooled_sb = outp.tile([B, OUT_DIM], F32)
            nc.vector.tensor_copy(pooled_sb[:], pool_ps[:])
            nc.sync.dma_start(pooled[:], pooled_sb[:])
    nc.compile()
    return nc


def _launch_b_tile_body(nc, wp, psS, psN, psD, isel_sb, pool_ps, t,
                        qgT_ap, kgT_ap, vg_ap, ind_ap, skip_ap,
                        indng_ap):
    q4 = qgT_ap.rearrange("p (ch cb e) -> p ch cb e", ch=CHUNKS, cb=4)
    k4 = kgT_ap.rearrange("p (ch cb e) -> p ch cb e", ch=CHUNKS, cb=4)

    if True:
            if True:
                num_ps = psN.tile([128, HC], F32, tag="num")
                den_ps = psD.tile([128, HEADS], F32, tag="den")
                for c0 in range(0, CHUNKS, 2):
                    w2 = min(2, CHUNKS - c0)
                    qkT = wp.tile([128, 2 * 512], BF16, tag="qkT")
                    qk4 = qkT[:].rearrange("p (w cb e) -> p w cb e", w=2, cb=4)
                    nc.vector.tensor_mul(
                        qk4[:, :w2], q4[:, c0:c0 + w2], k4[:, c0:c0 + w2])
                    s_ps = psS.tile([128, 2 * HEADS], F32, tag="s")
                    for j in range(w2):
                        for cb in range(4):
                            nc.tensor.matmul(
                                s_ps[:, j * HEADS + 2 * cb: j * HEADS + 2 * cb + 2],
                                lhsT=qk4[:, j, cb, :],
                                rhs=isel_sb[:],
                                start=True,
                                stop=True,
                            )
                    wnar = wp.tile([128, 2 * HEADS], BF16, tag="wnar")
                    nc.scalar.activation(
                        out=wnar[:, :w2 * HEADS],
                        in_=s_ps[:, :w2 * HEADS],
                        func=mybir.ActivationFunctionType.Exp,
                        scale=float(EXPSCALE),
                    )
                    wv = wp.tile([128, 2 * 512], BF16, tag="wv")
                    nc.vector.tensor_mul(
                        wv[:, :w2 * 512].rearrange(
                            "p (w h c) -> p w h c", w=w2, h=HEADS),
                        vg_ap[:, c0 * 512:(c0 + w2) * 512].rearrange(
                            "p (w h c) -> p w h c", w=w2, h=HEADS),
                        wnar[:, :w2 * HEADS].rearrange(
                            "p (w h) -> p w h ()", w=w2).to_broadcast(
                            [128, w2, HEADS, OUT_DIM]),
                    )
                    for j in range(w2):
                        c = c0 + j
                        nc.tensor.matmul(
                            num_ps[:], lhsT=ind_ap[:, ts(c, 128)], rhs=wv[:, ts(j, 512)],
                            start=(c == 0), stop=(c == CHUNKS - 1),
                        )
                        nc.tensor.matmul(
                            den_ps[:], lhsT=ind_ap[:, ts(c, 128)],
                            rhs=wnar[:, ts(j, HEADS)],
                            start=(c == 0), stop=(c == CHUNKS - 1),
                        )
                # epilogue: out = mean_h(num/den)/SQ + skip
                rec = wp.tile([128, HEADS], F32, tag="rec")
                nc.vector.tensor_scalar(
                    out=rec[:], in0=den_ps[:],
                    scalar1=float(HEADS * SQ), scalar2=1e-12,
                    op0=mybir.AluOpType.mult, op1=mybir.AluOpType.add,
                )
                nc.vector.reciprocal(rec[:], rec[:])
                recx = wp.tile([128, HC], BF16, tag="recx")
                nc.scalar.activation(
                    out=recx[:].rearrange("p (h c) -> p h c", h=HEADS),
                    in_=rec[:].rearrange("p h -> p h ()").to_broadcast(
                        [128, HEADS, OUT_DIM]),
                    func=mybir.ActivationFunctionType.Copy,
                )
                num_sb = wp.tile([128, HC], BF16, tag="numsb")
                nc.scalar.activation(
                    out=num_sb[:], in_=num_ps[:],
                    func=mybir.ActivationFunctionType.Copy,
                )
                mh = wp.tile([128, HC], BF16, tag="mh")
                nc.vector.tensor_mul(mh[:], num_sb[:], recx[:])
                f1 = wp.tile([128, 256], BF16, tag="f1")
                nc.vector.tensor_add(f1[:], mh[:, :256], mh[:, 256:])
                f2 = wp.tile([128, 128], BF16, tag="f2")
                nc.vector.tensor_add(f2[:], f1[:, :128], f1[:, 128:])
                f3 = wp.tile([128, 64], BF16, tag="f3")
                nc.vector.tensor_add(f3[:], f2[:, :64], f2[:, 64:])
                out_sb = wp.tile([128, OUT_DIM], BF16, tag="outsb")
                nc.vector.tensor_add(out_sb[:], f3[:], skip_ap[:])
                nc.tensor.matmul(
                    pool_ps[:], lhsT=indng_ap[:], rhs=out_sb[:],
                    start=(t == 0), stop=(t == TILES - 1),
                )


def _get_programs():
    if "A" not in _cache:
        _cache["A"] = _build_launch_a()
    if "B" not in _cache:
        _cache["B"] = _build_launch_b()
    return _cache["A"], _cache["B"]


LAST_TRACE_PATH = {}


def _ensure_hook_shim():
    import sys
    import types

    if "antenv.axon_hooks" in sys.modules:
        return
    mod = types.ModuleType("antenv.axon_hooks")
    holder = [None]
    mod.set_axon_ntff_profile_hook = lambda h: holder.__setitem__(0, h)
    mod.get_axon_ntff_profile_hook = lambda: holder[0]
    sys.modules["antenv.axon_hooks"] = mod
    import antenv

    antenv.axon_hooks = mod
    from trn_agent_boot.trn_boot import _ntff_profile_via_ctypes

    mod.set_axon_ntff_profile_hook(
        _ntff_profile_via_ctypes("/opt/axon/libaxon_pjrt.so")
    )


def _run(nc, in_maps, label):
    if not TRACE:
        res = bass_utils.run_bass_kernel_spmd(nc, in_maps, list(range(NCORES)))
        return res.results

    import glob
    import os
    import tempfile

    from concourse import bass2jax
    from concourse._compat import FishPath
    import gauge.profiler

    _ensure_hook_shim()
    import antenv.axon_hooks as hooks

    tmpdir = tempfile.mkdtemp(prefix=f"bass_{label}_")
    with hooks.get_axon_ntff_profile_hook()(tmpdir, [0]):
        results = bass2jax.run_bass_via_pjrt(nc, in_maps, n_cores=NCORES)
    exec_ns = None
    try:
        ntffs = glob.glob(os.path.join(tmpdir, "*_body*.ntff"))
        if ntffs:
            profile = gauge.profiler.Profile(
                profile_path=FishPath(tmpdir),
                kernel_dev_mode=True,
                profile_on_exit=False,
                bass_kernel=nc.m,
                offline_processing=True,
                fname="*_body*",
            )
            prs = profile.to_perfetto(model_index=(0,))
            if prs:
                exec_ns = max(p.exec_time_ns for p in prs)
                LAST_TRACE_PATH[label] = (tmpdir, [p.trace_path for p in prs])
        else:
            print(f"[{label}] no ntff files in {tmpdir}: {os.listdir(tmpdir)}")
    except Exception as e:  # profiling must never break the run
        print(f"[{label}] profile processing failed: {type(e).__name__}: {e}")
    LAST_EXEC_NS[label] = exec_ns
    return results


def _to_fp8(a):
    return np.clip(a, -240.0, 240.0).astype(NP_FP8)


def kernel(x, edge_index, batch, W_emb, b_emb, Wq, bq, Wk, bk, Wv, bv, Wskip, bskip):
    x = np.asarray(x, np.float32)
    edge_index = np.asarray(edge_index)
    batch_np = np.asarray(batch)
    ncA, ncB = _get_programs()

    # ---- host prep for launch A: fold W_emb/b_emb into the qkv/skip weights ----
    wemb_f = np.asarray(W_emb, np.float32)
    bemb_f = np.asarray(b_emb, np.float32)
    wqk = np.concatenate([np.asarray(Wq, np.float32), np.asarray(Wk, np.float32)], axis=1)
    wvs = np.concatenate([np.asarray(Wv, np.float32), np.asarray(Wskip, np.float32)], axis=1)
    bqk = np.concatenate([np.asarray(bq, np.float32), np.asarray(bk, np.float32)])
    bvs = np.concatenate([np.asarray(bv, np.float32), np.asarray(bskip, np.float32)])
    w8_f = (wemb_f @ wqk) * SW  # [768, 1024]
    w16_f = wemb_f @ wvs  # [768, 576]
    b8 = bemb_f @ wqk + bqk  # [1024]
    b16 = bemb_f @ wvs + bvs  # [576]

    w8_np = np.ascontiguousarray(
        _to_fp8(w8_f).reshape(KCH, 128, 1024).transpose(1, 0, 2))
    w16_np = np.ascontiguousarray(
        w16_f.astype(NP_BF16).reshape(KCH, 128, 576).transpose(1, 0, 2))
    bT_np = np.zeros((128, 13), np.float32)
    for i in range(8):
        bT_np[:, i] = SQ * b8[i * 128:(i + 1) * 128]
    for i in range(4):
        bT_np[:, 8 + i] = SQ * b16[i * 128:(i + 1) * 128]
    bT_np[:64, 12] = b16[512:576]

    # ---- degree-balanced node -> (core, tile, slot) assignment ----
    # snake-pack nodes by in-degree over all 400 tiles so every tile's edge
    # load fits CHUNKS*128 slots; repair pass for rare overloads
    src = np.asarray(edge_index[0], np.int64)
    dst = np.asarray(edge_index[1], np.int64)
    deg = np.bincount(dst, minlength=N).astype(np.int64)
    ntile = NCORES * TILES  # 400
    order_n = np.argsort(-deg, kind="stable")
    ridx = np.arange(N) // ntile
    posr = np.arange(N) % ntile
    tile_sorted = np.where(ridx % 2 == 0, posr, ntile - 1 - posr)
    tile_of_node = np.empty(N, np.int64)
    tile_of_node[order_n] = tile_sorted
    loads = np.bincount(tile_of_node, weights=deg.astype(np.float64),
                        minlength=ntile).astype(np.int64)
    cnt_t = np.bincount(tile_of_node, minlength=ntile)
    for _ in range(2000):
        tmax = int(loads.argmax())
        if loads[tmax] <= CAP:
            break
        tmin = int(np.where(cnt_t < 128, loads, np.iinfo(np.int64).max).argmin())
        members = np.where(tile_of_node == tmax)[0]
        need = loads[tmax] - CAP
        dm = deg[members]
        ok = dm >= need
        nmove = members[np.where(ok, dm, np.iinfo(np.int64).max).argmin()] \
            if ok.any() else members[dm.argmax()]
        tile_of_node[nmove] = tmin
        loads[tmax] -= deg[nmove]
        loads[tmin] += deg[nmove]
        cnt_t[tmax] -= 1
        cnt_t[tmin] += 1
    order2 = np.argsort(tile_of_node, kind="stable")
    g_sorted = tile_of_node[order2]
    starts_t = np.searchsorted(g_sorted, np.arange(ntile))
    slot_sorted = np.arange(N) - starts_t[g_sorted]
    pos_sorted = (g_sorted // TILES) * NPAD + (g_sorted % TILES) * 128 + slot_sorted
    pos_of_node = np.empty(N, np.int64)
    pos_of_node[order2] = pos_sorted

    xflat = np.zeros((NCORES * NPAD, IN_DIM), np.float32)
    xflat[pos_of_node] = x
    xpad = xflat.reshape(NCORES, NPAD, IN_DIM)
    in_maps_a = []
    for c in range(NCORES):
        xT = xpad[c].T  # [768, 6272]
        xTk = xT.reshape(KCH, 128, NPAD).transpose(1, 0, 2)  # [128, 6, 6272]
        in_maps_a.append({
            "x8": np.ascontiguousarray(_to_fp8(xTk)),
            "x16": np.ascontiguousarray(xTk.astype(NP_BF16)),
            "w8": w8_np, "w16": w16_np, "bT": bT_np,
        })
    res_a = _run(ncA, in_maps_a, "A")

    # ---- host mid: assemble qT/kT/v/skip and build edge-sorted gathers ----
    # global padded node index: n -> (n // NPC) * NPAD + n % NPC
    qT8 = np.concatenate(
        [res_a[c]["hT8"][0:512] for c in range(NCORES)], axis=1)  # [512, 8*NPAD]
    kT8 = np.concatenate(
        [res_a[c]["hT8"][512:1024] for c in range(NCORES)], axis=1)
    v8 = np.concatenate(
        [res_a[c]["hT8"][1024:1536].T for c in range(NCORES)], axis=0)  # [8*NPAD, 512]

    dstp = pos_of_node[dst]
    loc = dstp % NPAD
    tile_g = (dstp // NPAD) * TILES + loc // 128  # 0 .. 8*50-1
    dloc = loc % 128
    order = np.argsort(tile_g, kind="stable")
    tg_s = tile_g[order]
    counts = np.bincount(tg_s, minlength=ntile)
    if counts.max() > CAP:
        raise RuntimeError(f"tile capacity exceeded: {counts.max()} > {CAP}")
    starts = np.zeros(ntile, np.int64)
    starts[1:] = np.cumsum(counts)[:-1]
    pos = np.arange(E) - starts[tg_s]
    rows = tg_s * CAP + pos  # slot in [ntile*CAP]

    srcp_pad = np.zeros(ntile * CAP, np.int64)
    srcp_pad[rows] = pos_of_node[src[order]]
    dst_pad = np.full(ntile * CAP, -1, np.int64)
    dst_pad[rows] = dloc[order]
    dstgp_pad = np.zeros(ntile * CAP, np.int64)
    dstgp_pad[rows] = dstp[order]

    def t_gather(mT8, idx):  # [512, npad*8] cols idx [ntile_c*CAP] -> [49,128,4608]
        g = mT8[:, idx]  # [512, 56448]
        g = g.reshape(4, 128, TILES, CHUNKS, 128).transpose(2, 1, 3, 0, 4)
        return np.ascontiguousarray(g.reshape(TILES, 128, CHUNKS * 512))

    def e_gather(m8, idx):  # [npad*8, 512] rows idx -> [49, 128, 4608]
        g = m8[idx]  # [56448, 512]
        g = g.reshape(TILES, CHUNKS, 128, 512).transpose(0, 2, 1, 3)
        return np.ascontiguousarray(g.reshape(TILES, 128, CHUNKS * 512))

    def tileize(a):  # [ntile_c*CAP, d] -> [49, 128, CHUNKS*d]
        d = a.shape[1]
        return np.ascontiguousarray(
            a.reshape(TILES, CHUNKS, 128, d).transpose(0, 2, 1, 3).reshape(
                TILES, 128, CHUNKS * d))

    batch_pad = np.full(NCORES * NPAD, -1, np.int64)
    batch_pad[pos_of_node] = batch_np
    indng_all = (batch_pad[:, None] == np.arange(B)[None, :]).astype(NP_BF16)
    indng_all = indng_all.reshape(NCORES, TILES, 128, B)

    isel_np = np.zeros((128, 2), NP_BF16)
    isel_np[:64, 0] = 1
    isel_np[64:, 1] = 1

    in_maps_b = []
    for c in range(NCORES):
        sl = slice(c * TILES * CAP, (c + 1) * TILES * CAP)
        skip_c = np.ascontiguousarray(
            res_a[c]["skipT"].T.reshape(TILES, 128, OUT_DIM))
        in_maps_b.append({
            "qgT8": t_gather(qT8, dstgp_pad[sl]),
            "kgT8": t_gather(kT8, srcp_pad[sl]),
            "vg8": e_gather(v8, srcp_pad[sl]),
            "ind8": tileize(
                (dst_pad[sl, None] == np.arange(128)[None, :]).astype(NP_FP8)),
            "skip16": skip_c,
            "indng16": indng_all[c],
            "isel16": isel_np,
        })
    res_b = _run(ncB, in_maps_b, "B")

    pooled = np.zeros((B, OUT_DIM), np.float64)
    for c in range(NCORES):
        pooled += res_b[c]["pooled"].astype(np.float64)
    cnt = np.bincount(batch_np, minlength=B).astype(np.float64)
    pooled /= np.maximum(cnt, 1.0)[:, None]
    return pooled.astype(np.float32)
